# revision 29
# baseline (speedup 1.0000x reference)
"""Trainium2 Bass kernel for nn_AttentiveTransformer (Dense + BN + prior mask + sparsemax).

Strategy (data-parallel over 8 NeuronCores, batch sharded; fp16 IO):
  per 128-row tile (groups of 16 tiles):
    PE:   fp16 matmul x_tile^T @ W' (+bias via K=1 matmul) into [128,512] PSUM pairs
    DVE:  z = psum * priors (fp16 out);  top-8 per row via one max8 op
    GP:   tau8 recursion on the 16x8 sorted candidates (segmented scan trick)
    ACT:  out' = relu(z - tau8) with fused accumulator s = sum(out')  [one pass]
    DVE:  secant correction  tau_sec = tau8 + (s-1)(z8-tau8)/(s-s8), clamped
    GP:   out = relu(out' - (tau_sec - tau8))   [exact: relu(relu(a)-d)=relu(a-d), d>=0]
  tau8 <= tau* always (k-capped support), so s >= 1 and the bracket
  (tau8, s) x (z8, s(z8)) contains tau*; one secant step gives ~3e-3 max err.
"""
import os
import sys

sys.path.insert(0, "/opt/trn_rl_repo")

import numpy as np

import concourse.bass as bass
import concourse.mybir as mybir
from concourse.tile import TileContext

F32 = mybir.dt.float32
F16 = mybir.dt.float16
ALU = mybir.AluOpType
ACTF = mybir.ActivationFunctionType

N_CORES = 8
B = 262144
D_IN = 128
D_OUT = 256
BC = B // N_CORES          # rows per core
GSIZE = int(os.environ.get("K_GSIZE", "16"))  # tiles per stats group

# knobs (env-tunable for experiments)
RELU_ENG = os.environ.get("K_RELU_ENG", "dve")   # gp | dve
SCAN_ENG = os.environ.get("K_SCAN_ENG", "dve")   # gp | dve
SEC_ENG = os.environ.get("K_SEC_ENG", "dve")     # dve | gp
RELU_ACT_N = int(os.environ.get("K_RELU_ACT_N", "10"))  # tiles/group relu'd on ACT
MULT_QUAD = int(os.environ.get("K_MULT_QUAD", "1"))     # 4-tile psum mult


def _split_oversized_waits(nc, max_waits=1):
    """walrus setupSyncWait rejects instructions with many sem waits; split
    the excess onto same-engine Drain instructions placed just before."""
    for f in nc.m.functions:
        for bb in f.blocks:
            insts = bb.instructions
            i = 0
            while i < len(insts):
                inst = insts[i]
                si = inst.sync_info
                waits = list(si.on_wait) if si and si.on_wait else []
                if len(waits) > max_waits:
                    si.on_wait = waits[:max_waits]
                    rest = waits[max_waits:]
                    pos = i
                    for j in range(0, len(rest), max_waits):
                        d = mybir.InstDrain(
                            name=f"{inst.name}_wsplit{j}", ins=[], outs=[],
                            bass_is_fusable=False,
                        )
                        d.engine = inst.engine
                        d.sync_info = mybir.SyncInfo(
                            on_wait=rest[j:j + max_waits], on_update=[])
                        insts.insert(pos, d)
                        pos += 1
                        i += 1
                i += 1


def build_nc(bc=BC, reps=1, split_waits=True, debug=False):
    assert bc % 128 == 0
    n_tiles = bc // 128
    assert n_tiles % GSIZE == 0
    n_groups = n_tiles // GSIZE

    nc = bass.Bass()
    if debug:
        dbg_sg = nc.declare_dram_parameter("dbg_sg", [n_groups, 128, GSIZE], F32, isOutput=True)
        dbg_ntau = nc.declare_dram_parameter("dbg_ntau", [n_groups, 128, GSIZE], F32, isOutput=True)
        dbg_dneg = nc.declare_dram_parameter("dbg_dneg", [n_groups, 128, GSIZE], F32, isOutput=True)
        dbg_stats = nc.declare_dram_parameter("dbg_stats", [n_groups, 128, GSIZE * 8], F32, isOutput=True)
        dbg_z = nc.declare_dram_parameter("dbg_z", [n_groups, 128, GSIZE * D_OUT], F16, isOutput=True)
    xin = nc.declare_dram_parameter("xin", [D_IN, bc], F16, isOutput=False)
    prin = nc.declare_dram_parameter("prin", [bc, D_OUT], F16, isOutput=False)
    wp = nc.declare_dram_parameter("wp", [D_IN, D_OUT], F16, isOutput=False)
    bp = nc.declare_dram_parameter("bp", [1, 4 * D_OUT], F16, isOutput=False)
    ones = nc.declare_dram_parameter("ones", [1, D_IN], F16, isOutput=False)
    jc = nc.declare_dram_parameter("jc", [128, GSIZE * 8], F32, isOutput=False)
    sm = nc.declare_dram_parameter("sm", [128, GSIZE * 8], F32, isOutput=False)
    out = nc.declare_dram_parameter("out", [bc, D_OUT], F16, isOutput=True)

    # group-supertile views; priors/out: row (t*128 + p) -> [p, t, :]
    xin_g = xin[:, :].rearrange("d (g c) -> g d c", c=GSIZE * 128)
    prin_g = prin[:, :].rearrange("(g t p) d -> g p t d", p=128, t=GSIZE)
    out_g = out[:, :].rearrange("(g t p) d -> g p t d", p=128, t=GSIZE)

    with TileContext(nc) as tc:
        with (
            tc.tile_pool(name="const", bufs=1) as constp,
            tc.tile_pool(name="xload", bufs=2) as xloadp,
            tc.tile_pool(name="pload", bufs=2) as ploadp,
            tc.tile_pool(name="z", bufs=2) as zp,
            tc.tile_pool(name="outp", bufs=2) as outpp,
            tc.tile_pool(name="outs", bufs=2) as outsp,
            tc.tile_pool(name="stats", bufs=2) as statsp,
            tc.tile_pool(name="small", bufs=2) as smallp,
            tc.tile_pool(name="psz", bufs=(4 if MULT_QUAD else 6),
                         space="PSUM") as psumz,
        ):
            wp_sb = constp.tile([D_IN, D_OUT], F16)
            nc.sync.dma_start(out=wp_sb[:], in_=wp[:, :])
            bp_sb = constp.tile([1, 4 * D_OUT], F16)
            nc.sync.dma_start(out=bp_sb[:], in_=bp[:, :])
            ones_sb = constp.tile([1, D_IN], F16)
            nc.sync.dma_start(out=ones_sb[:], in_=ones[:, :])
            jc_sb = constp.tile([128, GSIZE * 8], F32)
            nc.sync.dma_start(out=jc_sb[:], in_=jc[:, :])
            sm_sb = constp.tile([128, GSIZE * 8], F32)
            nc.sync.dma_start(out=sm_sb[:], in_=sm[:, :])

            eng_scan = nc.gpsimd if SCAN_ENG == "gp" else nc.vector
            eng_sec = nc.gpsimd if SEC_ENG == "gp" else nc.vector
            eng_relu = nc.gpsimd if RELU_ENG == "gp" else nc.vector

            for g in range(n_groups * reps):
                g = g % n_groups
                xg = xloadp.tile([128, GSIZE * 128], F16)
                nc.sync.dma_start(out=xg[:], in_=xin_g[g])
                pg = ploadp.tile([128, GSIZE, D_OUT], F16)
                nc.sync.dma_start(out=pg[:], in_=prin_g[g])

                zg = zp.tile([128, GSIZE, D_OUT], F16)
                outp = outpp.tile([128, GSIZE, D_OUT], F16)
                og = outsp.tile([128, GSIZE, D_OUT], F16)
                stats = statsp.tile([128, GSIZE * 8], F32)

                cums = statsp.tile([128, GSIZE * 8], F32, tag="cums")
                conds = statsp.tile([128, GSIZE * 8], F32, tag="conds")
                scratch = statsp.tile([128, GSIZE * 8], F32, tag="scratch")
                ntau = smallp.tile([128, GSIZE], F32, tag="ntau")
                sg = smallp.tile([128, GSIZE], F32, tag="sg")
                den = smallp.tile([128, GSIZE], F32, tag="den")
                den2 = smallp.tile([128, GSIZE], F32, tag="den2")
                rec = smallp.tile([128, GSIZE], F32, tag="rec")
                gapn = smallp.tile([128, GSIZE], F32, tag="gapn")
                num = smallp.tile([128, GSIZE], F32, tag="num")
                km = smallp.tile([128, GSIZE], F32, tag="km")
                tmp2 = smallp.tile([128, GSIZE], F32, tag="tmp2")
                dneg = smallp.tile([128, GSIZE], F32, tag="dneg")

                # PE + DVE: matmuls into psum batches, mult, top-8
                BW = 4 if MULT_QUAD else 2  # tiles per psum batch
                for pr in range(GSIZE // BW):
                    ts = [BW * pr + i for i in range(BW)]
                    ps = psumz.tile([128, BW, D_OUT], F32)
                    for i, t in enumerate(ts):
                        nc.tensor.matmul(ps[:, i, :],
                                         xg[:, t * 128:(t + 1) * 128],
                                         wp_sb[:], start=True, stop=False)
                        nc.tensor.matmul(ps[:, i, :], ones_sb[:],
                                         bp_sb[:, :D_OUT], start=False,
                                         stop=True)
                    t0 = ts[0]
                    nc.vector.tensor_tensor(zg[:, t0:t0 + BW, :], ps[:],
                                            pg[:, t0:t0 + BW, :], ALU.mult)
                    for t in ts:
                        nc.vector.max(stats[:, 8 * t:8 * t + 8], zg[:, t, :])

                # tau8 on sorted top-8 candidates via the simplex-projection
                # identity tau = max_j (cumsum_j - 1)/j, evaluated with one
                # segmented add-scan + one segmented max-scan (shifted +16 so
                # the max-scan's zero segment-reset never wins).
                eng_scan.tensor_tensor_scan(
                    cums[:], sm_sb[:], stats[:], 0.0, ALU.mult, ALU.add)
                eng_scan.scalar_tensor_tensor(
                    scratch[:], cums[:], -1.0, jc_sb[:], ALU.add, ALU.mult)
                nc.vector.tensor_scalar(
                    scratch[:], scratch[:], 16.0, None, ALU.add)
                eng_scan.tensor_tensor_scan(
                    conds[:], sm_sb[:], scratch[:], 0.0, ALU.mult, ALU.max)
                tm8 = conds[:].rearrange("p (g j) -> p g j", j=8)[:, :, 7:8]
                cs8v = cums[:].rearrange("p (g j) -> p g j", j=8)[:, :, 7:8]
                z8v = stats[:].rearrange("p (g j) -> p g j", j=8)[:, :, 7:8]
                # ntau = -tau8 = 16 - tm8
                nc.vector.tensor_scalar(
                    ntau[:], tm8, -1.0, 16.0, ALU.mult, ALU.add)

                # ACT: out' = relu(z - tau8), s = rowsum(out') per tile
                for t in range(GSIZE):
                    nc.scalar.activation(
                        outp[:, t, :], zg[:, t, :], ACTF.Relu,
                        bias=ntau[:, t:t + 1], scale=1.0,
                        accum_out=sg[:, t:t + 1])

                # secant: dneg = -(s-1)^+ * (z8 - tau8) / max(s - s8, eps)
                # den = s - s8 = sg - cs8 + 8*z8
                eng_sec.scalar_tensor_tensor(
                    den[:], cs8v, -1.0, sg[:], ALU.mult, ALU.add)    # sg - cs8
                eng_sec.scalar_tensor_tensor(
                    den2[:], z8v, 8.0, den[:], ALU.mult, ALU.add)     # + 8*z8
                nc.vector.tensor_scalar(den2[:], den2[:], 1e-6, None, ALU.max)
                nc.vector.reciprocal(rec[:], den2[:])
                # gap_neg = tau8 - z8 = (-z8) - ntau8
                eng_sec.scalar_tensor_tensor(
                    gapn[:], z8v, -1.0, ntau[:], ALU.mult, ALU.subtract)
                eng_sec.tensor_scalar(num[:], sg[:], -1.0, 0.0, ALU.add, ALU.max)
                # gate: correction only valid when k8 == 8 <=> z8 > tau8
                eng_sec.tensor_scalar(km[:], gapn[:], 0.0, None, ALU.is_lt)
                eng_sec.tensor_tensor(num[:], num[:], km[:], ALU.mult)
                eng_sec.tensor_tensor(tmp2[:], num[:], gapn[:], ALU.mult)
                eng_sec.tensor_tensor(tmp2[:], tmp2[:], rec[:], ALU.mult)
                eng_sec.tensor_scalar(dneg[:], tmp2[:], 0.0, None, ALU.min)

                # final: out = relu(out' + dneg); split tiles ACT/DVE to balance
                for t in range(GSIZE):
                    if t < RELU_ACT_N:
                        nc.scalar.activation(
                            og[:, t, :], outp[:, t, :], ACTF.Relu,
                            bias=dneg[:, t:t + 1], scale=1.0)
                    else:
                        eng_relu.tensor_scalar(
                            og[:, t, :], outp[:, t, :], dneg[:, t:t + 1], 0.0,
                            ALU.add, ALU.max)
                nc.sync.dma_start(out=out_g[g], in_=og[:])
                if debug:
                    nc.sync.dma_start(out=dbg_sg[g], in_=sg[:])
                    nc.sync.dma_start(out=dbg_ntau[g], in_=ntau[:])
                    nc.sync.dma_start(out=dbg_dneg[g], in_=dneg[:])
                    nc.sync.dma_start(out=dbg_stats[g], in_=stats[:])
                    nc.sync.dma_start(out=dbg_z[g], in_=zg[:])

    if split_waits:
        _split_oversized_waits(nc)
    return nc


def _host_constants(W, gamma, beta, moving_mean, moving_var):
    inv = (gamma / np.sqrt(moving_var + 1e-3)).astype(np.float32)
    wp = (W * inv[None, :]).astype(np.float16)
    bpf = np.tile((beta - moving_mean * inv).astype(np.float16), 4).reshape(1, 4 * D_OUT)
    ones = np.ones((1, D_IN), dtype=np.float16)
    jrow = np.tile(1.0 / np.arange(1, 9, dtype=np.float32), GSIZE)
    jc = np.broadcast_to(jrow, (128, GSIZE * 8)).copy()
    srow = np.tile(
        np.concatenate([[0.0], np.ones(7, dtype=np.float32)]).astype(np.float32),
        GSIZE)
    sm = np.broadcast_to(srow, (128, GSIZE * 8)).copy()
    return wp, bpf, ones, jc, sm


_NC_CACHE = {}


def make_core_feeds(inputs, priors, W, gamma, beta, moving_mean, moving_var,
                    bc=BC, n_cores=N_CORES):
    inputs = np.asarray(inputs, dtype=np.float32)
    priors16 = np.ascontiguousarray(np.asarray(priors).astype(np.float16))
    inputs_t = np.ascontiguousarray(inputs.T.astype(np.float16))  # [D_IN, B]
    wp, bpf, ones, jc, sm = _host_constants(
        np.asarray(W, dtype=np.float32), np.asarray(gamma, dtype=np.float32),
        np.asarray(beta, dtype=np.float32),
        np.asarray(moving_mean, dtype=np.float32),
        np.asarray(moving_var, dtype=np.float32))
    in_maps = []
    for c in range(n_cores):
        lo, hi = c * bc, (c + 1) * bc
        in_maps.append({
            "xin": np.ascontiguousarray(inputs_t[:, lo:hi]),
            "prin": priors16[lo:hi],
            "wp": wp, "bp": bpf, "ones": ones, "jc": jc, "sm": sm,
        })
    return in_maps


def kernel(inputs, priors, W, gamma, beta, moving_mean, moving_var):
    from concourse.bass_utils import run_bass_kernel_spmd

    in_maps = make_core_feeds(inputs, priors, W, gamma, beta,
                              moving_mean, moving_var)
    if BC not in _NC_CACHE:
        _NC_CACHE[BC] = build_nc(BC)
    nc = _NC_CACHE[BC]
    res = run_bass_kernel_spmd(nc, in_maps, list(range(N_CORES)))
    return np.concatenate(
        [res.results[c]["out"].astype(np.float32) for c in range(N_CORES)],
        axis=0)


# revision 33
# speedup vs baseline: 1.1138x; 1.1138x over previous
"""Trainium2 Bass kernel for nn_AttentiveTransformer (Dense + BN + prior mask + sparsemax).

Strategy (data-parallel over 8 NeuronCores, batch sharded; fp16 IO):
  per 128-row tile (groups of 16 tiles):
    PE:   fp16 matmul x_tile^T @ W' (+bias via K=1 matmul) into [128,512] PSUM pairs
    DVE:  z = psum * priors (fp16 out);  top-8 per row via one max8 op
    GP:   tau8 recursion on the 16x8 sorted candidates (segmented scan trick)
    ACT:  out' = relu(z - tau8) with fused accumulator s = sum(out')  [one pass]
    DVE:  secant correction  tau_sec = tau8 + (s-1)(z8-tau8)/(s-s8), clamped
    GP:   out = relu(out' - (tau_sec - tau8))   [exact: relu(relu(a)-d)=relu(a-d), d>=0]
  tau8 <= tau* always (k-capped support), so s >= 1 and the bracket
  (tau8, s) x (z8, s(z8)) contains tau*; one secant step gives ~3e-3 max err.
"""
import os
import sys

sys.path.insert(0, "/opt/trn_rl_repo")

import numpy as np

import concourse.bass as bass
import concourse.mybir as mybir
from concourse.tile import TileContext

F32 = mybir.dt.float32
F16 = mybir.dt.float16
ALU = mybir.AluOpType
ACTF = mybir.ActivationFunctionType

N_CORES = 8
B = 262144
D_IN = 128
D_OUT = 256
BC = B // N_CORES          # rows per core
GSIZE = int(os.environ.get("K_GSIZE", "16"))  # tiles per stats group

# knobs (env-tunable for experiments)
RELU_ENG = os.environ.get("K_RELU_ENG", "dve")   # gp | dve
SCAN_ENG = os.environ.get("K_SCAN_ENG", "dve")   # gp | dve
SEC_ENG = os.environ.get("K_SEC_ENG", "dve")     # dve | gp
RELU_ACT_N = int(os.environ.get("K_RELU_ACT_N", "10"))  # tiles/group relu'd on ACT
MULT_QUAD = int(os.environ.get("K_MULT_QUAD", "1"))     # 4-tile psum mult


def _split_oversized_waits(nc, max_waits=1):
    """walrus setupSyncWait rejects instructions with many sem waits; split
    the excess onto same-engine Drain instructions placed just before."""
    for f in nc.m.functions:
        for bb in f.blocks:
            insts = bb.instructions
            i = 0
            while i < len(insts):
                inst = insts[i]
                si = inst.sync_info
                waits = list(si.on_wait) if si and si.on_wait else []
                if len(waits) > max_waits:
                    si.on_wait = waits[:max_waits]
                    rest = waits[max_waits:]
                    pos = i
                    for j in range(0, len(rest), max_waits):
                        d = mybir.InstDrain(
                            name=f"{inst.name}_wsplit{j}", ins=[], outs=[],
                            bass_is_fusable=False,
                        )
                        d.engine = inst.engine
                        d.sync_info = mybir.SyncInfo(
                            on_wait=rest[j:j + max_waits], on_update=[])
                        insts.insert(pos, d)
                        pos += 1
                        i += 1
                i += 1


def build_nc(bc=BC, reps=1, split_waits=True, debug=False):
    assert bc % 128 == 0
    n_tiles = bc // 128
    assert n_tiles % GSIZE == 0
    n_groups = n_tiles // GSIZE

    nc = bass.Bass()
    if debug:
        dbg_sg = nc.declare_dram_parameter("dbg_sg", [n_groups, 128, GSIZE], F32, isOutput=True)
        dbg_ntau = nc.declare_dram_parameter("dbg_ntau", [n_groups, 128, GSIZE], F32, isOutput=True)
        dbg_dneg = nc.declare_dram_parameter("dbg_dneg", [n_groups, 128, GSIZE], F32, isOutput=True)
        dbg_stats = nc.declare_dram_parameter("dbg_stats", [n_groups, 128, GSIZE * 8], F32, isOutput=True)
        dbg_z = nc.declare_dram_parameter("dbg_z", [n_groups, 128, GSIZE * D_OUT], F16, isOutput=True)
    xin = nc.declare_dram_parameter("xin", [D_IN, bc], F16, isOutput=False)
    prin = nc.declare_dram_parameter("prin", [bc, D_OUT], F16, isOutput=False)
    wp = nc.declare_dram_parameter("wp", [D_IN, D_OUT], F16, isOutput=False)
    bp = nc.declare_dram_parameter("bp", [1, 4 * D_OUT], F16, isOutput=False)
    ones = nc.declare_dram_parameter("ones", [1, D_IN], F16, isOutput=False)
    jc = nc.declare_dram_parameter("jc", [128, GSIZE * 8], F32, isOutput=False)
    sm = nc.declare_dram_parameter("sm", [128, GSIZE * 8], F32, isOutput=False)
    out = nc.declare_dram_parameter("out", [bc, D_OUT], F16, isOutput=True)

    # group-supertile views; priors/out: row (t*128 + p) -> [p, t, :]
    xin_g = xin[:, :].rearrange("d (g c) -> g d c", c=GSIZE * 128)
    prin_g = prin[:, :].rearrange("(g t p) d -> g p t d", p=128, t=GSIZE)
    out_g = out[:, :].rearrange("(g t p) d -> g p t d", p=128, t=GSIZE)

    with TileContext(nc) as tc:
        with (
            tc.tile_pool(name="const", bufs=1) as constp,
            tc.tile_pool(name="xload", bufs=2) as xloadp,
            tc.tile_pool(name="pload", bufs=2) as ploadp,
            tc.tile_pool(name="z", bufs=2) as zp,
            tc.tile_pool(name="outp", bufs=2) as outpp,
            tc.tile_pool(name="outs", bufs=2) as outsp,
            tc.tile_pool(name="stats", bufs=2) as statsp,
            tc.tile_pool(name="small", bufs=2) as smallp,
            tc.tile_pool(name="psz", bufs=(4 if MULT_QUAD else 6),
                         space="PSUM") as psumz,
        ):
            wp_sb = constp.tile([D_IN, D_OUT], F16)
            nc.sync.dma_start(out=wp_sb[:], in_=wp[:, :])
            bp_sb = constp.tile([1, 4 * D_OUT], F16)
            nc.sync.dma_start(out=bp_sb[:], in_=bp[:, :])
            ones_sb = constp.tile([1, D_IN], F16)
            nc.sync.dma_start(out=ones_sb[:], in_=ones[:, :])
            jc_sb = constp.tile([128, GSIZE * 8], F32)
            nc.sync.dma_start(out=jc_sb[:], in_=jc[:, :])
            sm_sb = constp.tile([128, GSIZE * 8], F32)
            nc.sync.dma_start(out=sm_sb[:], in_=sm[:, :])

            eng_scan = nc.gpsimd if SCAN_ENG == "gp" else nc.vector
            eng_sec = nc.gpsimd if SEC_ENG == "gp" else nc.vector
            eng_relu = nc.gpsimd if RELU_ENG == "gp" else nc.vector

            for g in range(n_groups * reps):
                g = g % n_groups
                xg = xloadp.tile([128, GSIZE * 128], F16)
                nc.sync.dma_start(out=xg[:], in_=xin_g[g])
                pg = ploadp.tile([128, GSIZE, D_OUT], F16)
                nc.sync.dma_start(out=pg[:], in_=prin_g[g])

                zg = zp.tile([128, GSIZE, D_OUT], F16)
                outp = outpp.tile([128, GSIZE, D_OUT], F16)
                og = outsp.tile([128, GSIZE, D_OUT], F16)
                stats = statsp.tile([128, GSIZE * 8], F32)

                cums = statsp.tile([128, GSIZE * 8], F32, tag="cums")
                conds = statsp.tile([128, GSIZE * 8], F32, tag="conds")
                scratch = statsp.tile([128, GSIZE * 8], F32, tag="scratch")
                kcum = statsp.tile([128, GSIZE * 8], F32, tag="kcum")
                scum = statsp.tile([128, GSIZE * 8], F32, tag="scum")
                rk = smallp.tile([128, GSIZE], F32, tag="rk")
                kneg = smallp.tile([128, GSIZE], F32, tag="kneg")
                ntau = smallp.tile([128, GSIZE], F32, tag="ntau")
                sg = smallp.tile([128, GSIZE], F32, tag="sg")
                den = smallp.tile([128, GSIZE], F32, tag="den")
                den2 = smallp.tile([128, GSIZE], F32, tag="den2")
                rec = smallp.tile([128, GSIZE], F32, tag="rec")
                gapn = smallp.tile([128, GSIZE], F32, tag="gapn")
                num = smallp.tile([128, GSIZE], F32, tag="num")
                km = smallp.tile([128, GSIZE], F32, tag="km")
                tmp2 = smallp.tile([128, GSIZE], F32, tag="tmp2")
                dneg = smallp.tile([128, GSIZE], F32, tag="dneg")

                # PE + DVE: matmuls into psum batches, mult, top-8
                BW = 4 if MULT_QUAD else 2  # tiles per psum batch
                for pr in range(GSIZE // BW):
                    ts = [BW * pr + i for i in range(BW)]
                    ps = psumz.tile([128, BW, D_OUT], F32)
                    for i, t in enumerate(ts):
                        nc.tensor.matmul(ps[:, i, :],
                                         xg[:, t * 128:(t + 1) * 128],
                                         wp_sb[:], start=True, stop=False)
                        nc.tensor.matmul(ps[:, i, :], ones_sb[:],
                                         bp_sb[:, :D_OUT], start=False,
                                         stop=True)
                    t0 = ts[0]
                    nc.vector.tensor_tensor(zg[:, t0:t0 + BW, :], ps[:],
                                            pg[:, t0:t0 + BW, :], ALU.mult)
                    for t in ts:
                        nc.vector.max(stats[:, 8 * t:8 * t + 8], zg[:, t, :])

                # tau8 recursion on sorted top-8 candidates (whole group).
                # All per-tile reductions come from segmented prefix scans
                # read at in-tile position 7 (strided views), no tensor_reduce.
                eng_scan.tensor_tensor_scan(
                    cums[:], sm_sb[:], stats[:], 0.0, ALU.mult, ALU.add)
                eng_scan.tensor_tensor(scratch[:], stats[:], jc_sb[:], ALU.mult)
                eng_scan.scalar_tensor_tensor(
                    conds[:], scratch[:], 1.0, cums[:], ALU.add, ALU.is_gt)
                eng_scan.tensor_tensor_scan(
                    kcum[:], sm_sb[:], conds[:], 0.0, ALU.mult, ALU.add)
                eng_scan.tensor_tensor(scratch[:], conds[:], stats[:], ALU.mult)
                eng_scan.tensor_tensor_scan(
                    scum[:], sm_sb[:], scratch[:], 0.0, ALU.mult, ALU.add)
                kg8 = kcum[:].rearrange("p (g j) -> p g j", j=8)[:, :, 7:8]
                stg8 = scum[:].rearrange("p (g j) -> p g j", j=8)[:, :, 7:8]
                cs8v = cums[:].rearrange("p (g j) -> p g j", j=8)[:, :, 7:8]
                z8v = stats[:].rearrange("p (g j) -> p g j", j=8)[:, :, 7:8]
                nc.vector.tensor_scalar(kneg[:], kg8, -1.0, None, ALU.mult)
                nc.vector.reciprocal(rk[:], kneg[:])
                nc.vector.scalar_tensor_tensor(
                    ntau[:], stg8, 1.0, rk[:], ALU.subtract, ALU.mult)

                # ACT: out' = relu(z - tau8), s = rowsum(out') per tile
                for t in range(GSIZE):
                    nc.scalar.activation(
                        outp[:, t, :], zg[:, t, :], ACTF.Relu,
                        bias=ntau[:, t:t + 1], scale=1.0,
                        accum_out=sg[:, t:t + 1])

                # secant: dneg = -(s-1)^+ * (z8 - tau8) / max(s - s8, eps)
                # den = s - s8 = sg - cs8 + 8*z8
                eng_sec.scalar_tensor_tensor(
                    den[:], cs8v, -1.0, sg[:], ALU.mult, ALU.add)    # sg - cs8
                eng_sec.scalar_tensor_tensor(
                    den2[:], z8v, 8.0, den[:], ALU.mult, ALU.add)     # + 8*z8
                nc.vector.tensor_scalar(den2[:], den2[:], 1e-6, None, ALU.max)
                nc.vector.reciprocal(rec[:], den2[:])
                # gap_neg = tau8 - z8 = (-z8) - ntau8
                eng_sec.scalar_tensor_tensor(
                    gapn[:], z8v, -1.0, ntau[:], ALU.mult, ALU.subtract)
                eng_sec.tensor_scalar(num[:], sg[:], -1.0, 0.0, ALU.add, ALU.max)
                # gate: correction only valid when k8 == 8 (else tau8 is exact)
                eng_sec.tensor_scalar(km[:], kg8, 7.5, None, ALU.is_gt)
                eng_sec.tensor_tensor(num[:], num[:], km[:], ALU.mult)
                eng_sec.tensor_tensor(tmp2[:], num[:], gapn[:], ALU.mult)
                eng_sec.tensor_tensor(tmp2[:], tmp2[:], rec[:], ALU.mult)
                eng_sec.tensor_scalar(dneg[:], tmp2[:], 0.0, None, ALU.min)

                # final: out = relu(out' + dneg); split tiles ACT/DVE to balance
                for t in range(GSIZE):
                    if t < RELU_ACT_N:
                        nc.scalar.activation(
                            og[:, t, :], outp[:, t, :], ACTF.Relu,
                            bias=dneg[:, t:t + 1], scale=1.0)
                    else:
                        eng_relu.tensor_scalar(
                            og[:, t, :], outp[:, t, :], dneg[:, t:t + 1], 0.0,
                            ALU.add, ALU.max)
                nc.sync.dma_start(out=out_g[g], in_=og[:])
                if debug:
                    nc.sync.dma_start(out=dbg_sg[g], in_=sg[:])
                    nc.sync.dma_start(out=dbg_ntau[g], in_=ntau[:])
                    nc.sync.dma_start(out=dbg_dneg[g], in_=dneg[:])
                    nc.sync.dma_start(out=dbg_stats[g], in_=stats[:])
                    nc.sync.dma_start(out=dbg_z[g], in_=zg[:])

    if split_waits:
        _split_oversized_waits(nc)
    return nc


def _host_constants(W, gamma, beta, moving_mean, moving_var):
    inv = (gamma / np.sqrt(moving_var + 1e-3)).astype(np.float32)
    wp = (W * inv[None, :]).astype(np.float16)
    bpf = np.tile((beta - moving_mean * inv).astype(np.float16), 4).reshape(1, 4 * D_OUT)
    ones = np.ones((1, D_IN), dtype=np.float16)
    jrow = np.tile(np.arange(1, 9, dtype=np.float32), GSIZE)
    jc = np.broadcast_to(jrow, (128, GSIZE * 8)).copy()
    srow = np.tile(
        np.concatenate([[0.0], np.ones(7, dtype=np.float32)]).astype(np.float32),
        GSIZE)
    sm = np.broadcast_to(srow, (128, GSIZE * 8)).copy()
    return wp, bpf, ones, jc, sm


_NC_CACHE = {}


def make_core_feeds(inputs, priors, W, gamma, beta, moving_mean, moving_var,
                    bc=BC, n_cores=N_CORES):
    inputs = np.asarray(inputs, dtype=np.float32)
    priors16 = np.ascontiguousarray(np.asarray(priors).astype(np.float16))
    inputs_t = np.ascontiguousarray(inputs.T.astype(np.float16))  # [D_IN, B]
    wp, bpf, ones, jc, sm = _host_constants(
        np.asarray(W, dtype=np.float32), np.asarray(gamma, dtype=np.float32),
        np.asarray(beta, dtype=np.float32),
        np.asarray(moving_mean, dtype=np.float32),
        np.asarray(moving_var, dtype=np.float32))
    in_maps = []
    for c in range(n_cores):
        lo, hi = c * bc, (c + 1) * bc
        in_maps.append({
            "xin": np.ascontiguousarray(inputs_t[:, lo:hi]),
            "prin": priors16[lo:hi],
            "wp": wp, "bp": bpf, "ones": ones, "jc": jc, "sm": sm,
        })
    return in_maps


def kernel(inputs, priors, W, gamma, beta, moving_mean, moving_var):
    from concourse.bass_utils import run_bass_kernel_spmd

    in_maps = make_core_feeds(inputs, priors, W, gamma, beta,
                              moving_mean, moving_var)
    if BC not in _NC_CACHE:
        _NC_CACHE[BC] = build_nc(BC)
    nc = _NC_CACHE[BC]
    res = run_bass_kernel_spmd(nc, in_maps, list(range(N_CORES)))
    return np.concatenate(
        [res.results[c]["out"].astype(np.float32) for c in range(N_CORES)],
        axis=0)


# revision 34
# speedup vs baseline: 1.1631x; 1.0443x over previous
"""Trainium2 Bass kernel for nn_AttentiveTransformer (Dense + BN + prior mask + sparsemax).

Strategy (data-parallel over 8 NeuronCores, batch sharded; fp16 IO):
  per 128-row tile (groups of 16 tiles):
    PE:   fp16 matmul x_tile^T @ W' (+bias via K=1 matmul) into [128,512] PSUM pairs
    DVE:  z = psum * priors (fp16 out);  top-8 per row via one max8 op
    GP:   tau8 recursion on the 16x8 sorted candidates (segmented scan trick)
    ACT:  out' = relu(z - tau8) with fused accumulator s = sum(out')  [one pass]
    DVE:  secant correction  tau_sec = tau8 + (s-1)(z8-tau8)/(s-s8), clamped
    GP:   out = relu(out' - (tau_sec - tau8))   [exact: relu(relu(a)-d)=relu(a-d), d>=0]
  tau8 <= tau* always (k-capped support), so s >= 1 and the bracket
  (tau8, s) x (z8, s(z8)) contains tau*; one secant step gives ~3e-3 max err.
"""
import os
import sys

sys.path.insert(0, "/opt/trn_rl_repo")

import numpy as np

import concourse.bass as bass
import concourse.mybir as mybir
from concourse.tile import TileContext

F32 = mybir.dt.float32
F16 = mybir.dt.float16
ALU = mybir.AluOpType
ACTF = mybir.ActivationFunctionType

N_CORES = 8
B = 262144
D_IN = 128
D_OUT = 256
BC = B // N_CORES          # rows per core
GSIZE = int(os.environ.get("K_GSIZE", "16"))  # tiles per stats group

# knobs (env-tunable for experiments)
RELU_ENG = os.environ.get("K_RELU_ENG", "dve")   # gp | dve
SCAN_ENG = os.environ.get("K_SCAN_ENG", "dve")   # gp | dve
SEC_ENG = os.environ.get("K_SEC_ENG", "dve")     # dve | gp
RELU_ACT_N = int(os.environ.get("K_RELU_ACT_N", "12"))  # tiles/group relu'd on ACT
MULT_QUAD = int(os.environ.get("K_MULT_QUAD", "1"))     # 4-tile psum mult


def _split_oversized_waits(nc, max_waits=1):
    """walrus setupSyncWait rejects instructions with many sem waits; split
    the excess onto same-engine Drain instructions placed just before."""
    for f in nc.m.functions:
        for bb in f.blocks:
            insts = bb.instructions
            i = 0
            while i < len(insts):
                inst = insts[i]
                si = inst.sync_info
                waits = list(si.on_wait) if si and si.on_wait else []
                if len(waits) > max_waits:
                    si.on_wait = waits[:max_waits]
                    rest = waits[max_waits:]
                    pos = i
                    for j in range(0, len(rest), max_waits):
                        d = mybir.InstDrain(
                            name=f"{inst.name}_wsplit{j}", ins=[], outs=[],
                            bass_is_fusable=False,
                        )
                        d.engine = inst.engine
                        d.sync_info = mybir.SyncInfo(
                            on_wait=rest[j:j + max_waits], on_update=[])
                        insts.insert(pos, d)
                        pos += 1
                        i += 1
                i += 1


def build_nc(bc=BC, reps=1, split_waits=True, debug=False):
    assert bc % 128 == 0
    n_tiles = bc // 128
    assert n_tiles % GSIZE == 0
    n_groups = n_tiles // GSIZE

    nc = bass.Bass()
    if debug:
        dbg_sg = nc.declare_dram_parameter("dbg_sg", [n_groups, 128, GSIZE], F32, isOutput=True)
        dbg_ntau = nc.declare_dram_parameter("dbg_ntau", [n_groups, 128, GSIZE], F32, isOutput=True)
        dbg_dneg = nc.declare_dram_parameter("dbg_dneg", [n_groups, 128, GSIZE], F32, isOutput=True)
        dbg_stats = nc.declare_dram_parameter("dbg_stats", [n_groups, 128, GSIZE * 8], F32, isOutput=True)
        dbg_z = nc.declare_dram_parameter("dbg_z", [n_groups, 128, GSIZE * D_OUT], F16, isOutput=True)
    xin = nc.declare_dram_parameter("xin", [D_IN, bc], F16, isOutput=False)
    prin = nc.declare_dram_parameter("prin", [bc, D_OUT], F16, isOutput=False)
    wp = nc.declare_dram_parameter("wp", [D_IN, D_OUT], F16, isOutput=False)
    bp = nc.declare_dram_parameter("bp", [1, 4 * D_OUT], F16, isOutput=False)
    ones = nc.declare_dram_parameter("ones", [1, D_IN], F16, isOutput=False)
    jc = nc.declare_dram_parameter("jc", [128, GSIZE * 8], F32, isOutput=False)
    sm = nc.declare_dram_parameter("sm", [128, GSIZE * 8], F32, isOutput=False)
    out = nc.declare_dram_parameter("out", [bc, D_OUT], F16, isOutput=True)

    # group-supertile views; priors/out: row (t*128 + p) -> [p, t, :]
    xin_g = xin[:, :].rearrange("d (g c) -> g d c", c=GSIZE * 128)
    prin_g = prin[:, :].rearrange("(g t p) d -> g p t d", p=128, t=GSIZE)
    out_g = out[:, :].rearrange("(g t p) d -> g p t d", p=128, t=GSIZE)

    with TileContext(nc) as tc:
        with (
            tc.tile_pool(name="const", bufs=1) as constp,
            tc.tile_pool(name="xload", bufs=2) as xloadp,
            tc.tile_pool(name="pload", bufs=2) as ploadp,
            tc.tile_pool(name="z", bufs=2) as zp,
            tc.tile_pool(name="outp", bufs=2) as outpp,
            tc.tile_pool(name="outs", bufs=2) as outsp,
            tc.tile_pool(name="stats", bufs=2) as statsp,
            tc.tile_pool(name="small", bufs=2) as smallp,
            tc.tile_pool(name="psz", bufs=(4 if MULT_QUAD else 6),
                         space="PSUM") as psumz,
        ):
            wp_sb = constp.tile([D_IN, D_OUT], F16)
            nc.sync.dma_start(out=wp_sb[:], in_=wp[:, :])
            bp_sb = constp.tile([1, 4 * D_OUT], F16)
            nc.sync.dma_start(out=bp_sb[:], in_=bp[:, :])
            ones_sb = constp.tile([1, D_IN], F16)
            nc.sync.dma_start(out=ones_sb[:], in_=ones[:, :])
            jc_sb = constp.tile([128, GSIZE * 8], F32)
            nc.sync.dma_start(out=jc_sb[:], in_=jc[:, :])
            sm_sb = constp.tile([128, GSIZE * 8], F32)
            nc.sync.dma_start(out=sm_sb[:], in_=sm[:, :])

            eng_scan = nc.gpsimd if SCAN_ENG == "gp" else nc.vector
            eng_sec = nc.gpsimd if SEC_ENG == "gp" else nc.vector
            eng_relu = nc.gpsimd if RELU_ENG == "gp" else nc.vector

            for g in range(n_groups * reps):
                g = g % n_groups
                xg = xloadp.tile([128, GSIZE * 128], F16)
                nc.sync.dma_start(out=xg[:], in_=xin_g[g])
                pg = ploadp.tile([128, GSIZE, D_OUT], F16)
                nc.sync.dma_start(out=pg[:], in_=prin_g[g])

                zg = zp.tile([128, GSIZE, D_OUT], F16)
                outp = outpp.tile([128, GSIZE, D_OUT], F16)
                og = outsp.tile([128, GSIZE, D_OUT], F16)
                stats = statsp.tile([128, GSIZE * 8], F32)

                cums = statsp.tile([128, GSIZE * 8], F32, tag="cums")
                conds = statsp.tile([128, GSIZE * 8], F32, tag="conds")
                scratch = statsp.tile([128, GSIZE * 8], F32, tag="scratch")
                kcum = statsp.tile([128, GSIZE * 8], F32, tag="kcum")
                scum = statsp.tile([128, GSIZE * 8], F32, tag="scum")
                rk = smallp.tile([128, GSIZE], F32, tag="rk")
                kneg = smallp.tile([128, GSIZE], F32, tag="kneg")
                ntau = smallp.tile([128, GSIZE], F32, tag="ntau")
                sg = smallp.tile([128, GSIZE], F32, tag="sg")
                den = smallp.tile([128, GSIZE], F32, tag="den")
                den2 = smallp.tile([128, GSIZE], F32, tag="den2")
                rec = smallp.tile([128, GSIZE], F32, tag="rec")
                gapn = smallp.tile([128, GSIZE], F32, tag="gapn")
                num = smallp.tile([128, GSIZE], F32, tag="num")
                km = smallp.tile([128, GSIZE], F32, tag="km")
                tmp2 = smallp.tile([128, GSIZE], F32, tag="tmp2")
                dneg = smallp.tile([128, GSIZE], F32, tag="dneg")

                # PE + DVE: matmuls into psum batches, mult, top-8
                BW = 4 if MULT_QUAD else 2  # tiles per psum batch
                for pr in range(GSIZE // BW):
                    ts = [BW * pr + i for i in range(BW)]
                    ps = psumz.tile([128, BW, D_OUT], F32)
                    for i, t in enumerate(ts):
                        nc.tensor.matmul(ps[:, i, :],
                                         xg[:, t * 128:(t + 1) * 128],
                                         wp_sb[:], start=True, stop=False)
                        nc.tensor.matmul(ps[:, i, :], ones_sb[:],
                                         bp_sb[:, :D_OUT], start=False,
                                         stop=True)
                    t0 = ts[0]
                    nc.vector.tensor_tensor(zg[:, t0:t0 + BW, :], ps[:],
                                            pg[:, t0:t0 + BW, :], ALU.mult)
                    for t in ts:
                        nc.vector.max(stats[:, 8 * t:8 * t + 8], zg[:, t, :])

                # tau8 recursion on sorted top-8 candidates (whole group).
                # All per-tile reductions come from segmented prefix scans
                # read at in-tile position 7 (strided views), no tensor_reduce.
                eng_scan.tensor_tensor_scan(
                    cums[:], sm_sb[:], stats[:], 0.0, ALU.mult, ALU.add)
                eng_scan.tensor_tensor(scratch[:], stats[:], jc_sb[:], ALU.mult)
                eng_scan.scalar_tensor_tensor(
                    conds[:], scratch[:], 1.0, cums[:], ALU.add, ALU.is_gt)
                eng_scan.tensor_tensor_scan(
                    kcum[:], sm_sb[:], conds[:], 0.0, ALU.mult, ALU.add)
                eng_scan.tensor_tensor(scratch[:], conds[:], stats[:], ALU.mult)
                eng_scan.tensor_tensor_scan(
                    scum[:], sm_sb[:], scratch[:], 0.0, ALU.mult, ALU.add)
                kg8 = kcum[:].rearrange("p (g j) -> p g j", j=8)[:, :, 7:8]
                stg8 = scum[:].rearrange("p (g j) -> p g j", j=8)[:, :, 7:8]
                cs8v = cums[:].rearrange("p (g j) -> p g j", j=8)[:, :, 7:8]
                z8v = stats[:].rearrange("p (g j) -> p g j", j=8)[:, :, 7:8]
                nc.vector.tensor_scalar(kneg[:], kg8, -1.0, None, ALU.mult)
                nc.vector.reciprocal(rk[:], kneg[:])
                nc.vector.scalar_tensor_tensor(
                    ntau[:], stg8, 1.0, rk[:], ALU.subtract, ALU.mult)

                # ACT: out' = relu(z - tau8), s = rowsum(out') per tile
                for t in range(GSIZE):
                    nc.scalar.activation(
                        outp[:, t, :], zg[:, t, :], ACTF.Relu,
                        bias=ntau[:, t:t + 1], scale=1.0,
                        accum_out=sg[:, t:t + 1])

                # secant: dneg = -(s-1)^+ * (z8 - tau8) / max(s - s8, eps)
                # den = s - s8 = sg - cs8 + 8*z8
                eng_sec.scalar_tensor_tensor(
                    den[:], cs8v, -1.0, sg[:], ALU.mult, ALU.add)    # sg - cs8
                eng_sec.scalar_tensor_tensor(
                    den2[:], z8v, 8.0, den[:], ALU.mult, ALU.add)     # + 8*z8
                nc.vector.tensor_scalar(den2[:], den2[:], 1e-6, None, ALU.max)
                nc.vector.reciprocal(rec[:], den2[:])
                # gap_neg = tau8 - z8 = (-z8) - ntau8
                eng_sec.scalar_tensor_tensor(
                    gapn[:], z8v, -1.0, ntau[:], ALU.mult, ALU.subtract)
                eng_sec.tensor_scalar(num[:], sg[:], -1.0, 0.0, ALU.add, ALU.max)
                # gate: correction only valid when k8 == 8 (else tau8 is exact)
                eng_sec.tensor_scalar(km[:], kg8, 7.5, None, ALU.is_gt)
                eng_sec.tensor_tensor(num[:], num[:], km[:], ALU.mult)
                eng_sec.tensor_tensor(tmp2[:], num[:], gapn[:], ALU.mult)
                eng_sec.tensor_tensor(tmp2[:], tmp2[:], rec[:], ALU.mult)
                eng_sec.tensor_scalar(dneg[:], tmp2[:], 0.0, None, ALU.min)

                # final: out = relu(out' + dneg); split tiles ACT/DVE to balance
                for t in range(GSIZE):
                    if t < RELU_ACT_N:
                        nc.scalar.activation(
                            og[:, t, :], outp[:, t, :], ACTF.Relu,
                            bias=dneg[:, t:t + 1], scale=1.0)
                    else:
                        eng_relu.tensor_scalar(
                            og[:, t, :], outp[:, t, :], dneg[:, t:t + 1], 0.0,
                            ALU.add, ALU.max)
                nc.sync.dma_start(out=out_g[g], in_=og[:])
                if debug:
                    nc.sync.dma_start(out=dbg_sg[g], in_=sg[:])
                    nc.sync.dma_start(out=dbg_ntau[g], in_=ntau[:])
                    nc.sync.dma_start(out=dbg_dneg[g], in_=dneg[:])
                    nc.sync.dma_start(out=dbg_stats[g], in_=stats[:])
                    nc.sync.dma_start(out=dbg_z[g], in_=zg[:])

    if split_waits:
        _split_oversized_waits(nc)
    return nc


def _host_constants(W, gamma, beta, moving_mean, moving_var):
    inv = (gamma / np.sqrt(moving_var + 1e-3)).astype(np.float32)
    wp = (W * inv[None, :]).astype(np.float16)
    bpf = np.tile((beta - moving_mean * inv).astype(np.float16), 4).reshape(1, 4 * D_OUT)
    ones = np.ones((1, D_IN), dtype=np.float16)
    jrow = np.tile(np.arange(1, 9, dtype=np.float32), GSIZE)
    jc = np.broadcast_to(jrow, (128, GSIZE * 8)).copy()
    srow = np.tile(
        np.concatenate([[0.0], np.ones(7, dtype=np.float32)]).astype(np.float32),
        GSIZE)
    sm = np.broadcast_to(srow, (128, GSIZE * 8)).copy()
    return wp, bpf, ones, jc, sm


_NC_CACHE = {}


def make_core_feeds(inputs, priors, W, gamma, beta, moving_mean, moving_var,
                    bc=BC, n_cores=N_CORES):
    inputs = np.asarray(inputs, dtype=np.float32)
    priors16 = np.ascontiguousarray(np.asarray(priors).astype(np.float16))
    inputs_t = np.ascontiguousarray(inputs.T.astype(np.float16))  # [D_IN, B]
    wp, bpf, ones, jc, sm = _host_constants(
        np.asarray(W, dtype=np.float32), np.asarray(gamma, dtype=np.float32),
        np.asarray(beta, dtype=np.float32),
        np.asarray(moving_mean, dtype=np.float32),
        np.asarray(moving_var, dtype=np.float32))
    in_maps = []
    for c in range(n_cores):
        lo, hi = c * bc, (c + 1) * bc
        in_maps.append({
            "xin": np.ascontiguousarray(inputs_t[:, lo:hi]),
            "prin": priors16[lo:hi],
            "wp": wp, "bp": bpf, "ones": ones, "jc": jc, "sm": sm,
        })
    return in_maps


def kernel(inputs, priors, W, gamma, beta, moving_mean, moving_var):
    from concourse.bass_utils import run_bass_kernel_spmd

    in_maps = make_core_feeds(inputs, priors, W, gamma, beta,
                              moving_mean, moving_var)
    if BC not in _NC_CACHE:
        _NC_CACHE[BC] = build_nc(BC)
    nc = _NC_CACHE[BC]
    res = run_bass_kernel_spmd(nc, in_maps, list(range(N_CORES)))
    return np.concatenate(
        [res.results[c]["out"].astype(np.float32) for c in range(N_CORES)],
        axis=0)
